# revision 17
# baseline (speedup 1.0000x reference)
"""Trainium2 Bass kernel for a 3-layer difflogic network (nn_Net_48610439856713).

Math: each layer o computes softmax(w[o])·ops16(a, b) with a = h[:, ia[o]],
b = h[:, ib[o]].  The 16 relaxed logic gates are all affine in {1, a, b, ab},
so the layer reduces to  h' = C0 + C1·a + C2·b + C3·a·b  with 4 per-neuron
coefficients derived on-device from softmax(w).

v3 design (vs the 2x4 baseline):
  - 8-way neuron sharding, full batch B=512 per core.  Layer exchange is ONE
    8-rank AllGather per layer (RDH algorithm, ~66us for 16MB out), which
    beats any 4-rank collective shape on this chip.
  - Layer 1 is computed as one-hot matmuls on the (otherwise idle) TensorE:
    a/b operand rows land in PSUM, removing layer-1's dma_gather descriptor
    generation (the Q7 SWDGE desc-gen at ~8ns/index is a main bottleneck).
  - Layer 2/3 gathers use prepare_only=True dma_gather: Q7 descriptor
    generation runs EARLY (overlapped with layer-1 compute / AllGather-1),
    and trigger_dma fires the prepared DMA once the AllGather has landed.
    A probe DMA reading the AG output + a WAW dep via signals_writable
    pins each trigger after its collective completes (the rust-side dep
    deferral alone gates only on the collective *doorbell*).
  - The per-neuron affine combine runs as 6 whole-layer DVE ops using
    stride-0 broadcast APs for the coefficient operands (instead of 3 small
    scalar_tensor_tensor ops per 128-slot column).

Host-side bookkeeping is integer/layout only: slot permutations, index
relabeling through the packed layout, int16 index wrapping, weight-row
packing, one-hot matrix construction.  All float arithmetic (softmax,
combine, sums) runs on device.
"""

import os
import numpy as np

P = 128
B = 512                  # full batch on every core
IN = 193
NGROUP = 3
TAU = 100.0
N_CORES = 8
SH = 8                   # neuron shards
BG = 1
BC = B                   # batch per core

NJ12 = 16                # j-columns per shard, layers 1/2
REAL12 = 2000            # real neurons per shard, layers 1/2
NS12 = NJ12 * P          # 2048 slots per shard

NJ3 = 18                 # layer 3: 3 groups x 6 j-cols
JPG = 6
NS3 = NJ3 * P            # 2304 slots
SPG = 15999 // NGROUP    # 5333 real neurons per group
CNT3 = [667, 667, 667, 667, 667, 666, 666, 666]   # per-shard split of 5333
OFF3 = np.concatenate([[0], np.cumsum(CNT3)[:-1]])

HD = os.environ.get("KERNEL_HDT", "f8")   # exchanged-activation dtype
SP = bool(int(os.environ.get("KERNEL_SP", "0")))  # single_packet gathers

_CACHE = {}


def _build_nc():
    import concourse.bacc as bacc
    import concourse.tile as tile
    import concourse.mybir as mybir

    f32 = mybir.dt.float32
    bf16 = mybir.dt.bfloat16
    i16 = mybir.dt.int16
    hdt = mybir.dt.float8e4 if HD == "f8" else bf16
    Alu = mybir.AluOpType
    Act = mybir.ActivationFunctionType
    Ax = mybir.AxisListType

    nc = bacc.Bacc("TRN2", target_bir_lowering=False, debug=False,
                   num_devices=N_CORES)

    G8 = [[0, 1, 2, 3, 4, 5, 6, 7]]

    # ---- I/O ----
    xT = nc.dram_tensor("xT", [IN, B], f32, kind="ExternalInput")
    oh0 = nc.dram_tensor("oh0", [P, 2 * NS12], hdt, kind="ExternalInput")
    oh1 = nc.dram_tensor("oh1", [IN - P, 2 * NS12], hdt, kind="ExternalInput")
    wps = [
        nc.dram_tensor("w1p", [P, NJ12 * 16], f32, kind="ExternalInput"),
        nc.dram_tensor("w2p", [P, NJ12 * 16], f32, kind="ExternalInput"),
        nc.dram_tensor("w3p", [P, NJ3 * 16], f32, kind="ExternalInput"),
    ]
    i2d = nc.dram_tensor("i2", [P, 2 * NS12 // 16], i16, kind="ExternalInput")
    i3d = nc.dram_tensor("i3", [P, 2 * NS3 // 16], i16, kind="ExternalInput")
    out_d = nc.dram_tensor("out", [1, NGROUP * B], f32, kind="ExternalOutput")

    # collective buffers: each layer's exchange is two half-AllGathers so the
    # first half ships while the second half is still combining.  g layout is
    # chunk-major: [2, SH*P, (NJ12/2)*B].
    JH = NJ12 // 2
    cin = [[nc.dram_tensor(f"cin{l}{k}", [P, JH * B], hdt, kind="Internal")
            for k in range(2)] for l in (1, 2)]
    gs_ = [nc.dram_tensor(f"g{l}", [2 * SH * P, JH * B], hdt, kind="Internal",
                          addr_space="Shared")
           for l in (1, 2)]
    win = nc.dram_tensor("win", [1, 16], f32, kind="Internal")
    warm = nc.dram_tensor("warm", [8, 16], f32, kind="Internal",
                          addr_space="Shared")
    pin = nc.dram_tensor("pin", [1, NGROUP * B], f32, kind="Internal")
    pall = nc.dram_tensor("pall", [8, NGROUP * B], f32, kind="Internal",
                          addr_space="Shared")

    def coeffs(pool, wp, nj, li):
        """softmax(w) -> affine coefficients C0..C3, each [P, nj] f32."""
        tg = lambda name: f"{name}_{li}"
        wt = pool.tile([P, nj * 16], f32, tag=tg("wt"))
        nc.sync.dma_start(wt[:], wp[:])
        e = pool.tile([P, nj * 16], f32, tag=tg("e"))
        nc.scalar.activation(e[:], wt[:], Act.Exp)
        e3 = e[:].rearrange("p (j g) -> p j g", g=16)
        e4 = e[:].rearrange("p (j h q) -> p j h q", h=4, q=4)

        ssum = pool.tile([P, nj], f32, tag=tg("ssum"))
        nc.vector.reduce_sum(ssum[:], e3, axis=Ax.X)
        r = pool.tile([P, nj], f32, tag=tg("r"))
        nc.vector.reciprocal(r[:], ssum[:])

        c0 = pool.tile([P, nj], f32, tag=tg("c0"))
        c1 = pool.tile([P, nj], f32, tag=tg("c1"))
        c2 = pool.tile([P, nj], f32, tag=tg("c2"))
        c3 = pool.tile([P, nj], f32, tag=tg("c3"))

        nc.vector.reduce_sum(c0[:], e4[:, :, 2:4, :], axis=Ax.XY)
        t1 = pool.tile([P, nj], f32, tag=tg("t1"))
        t2 = pool.tile([P, nj], f32, tag=tg("t2"))
        nc.vector.reduce_sum(t1[:], e4[:, :, 0:2, 2:4], axis=Ax.XY)
        nc.vector.reduce_sum(t2[:], e4[:, :, 2:4, 0:2], axis=Ax.XY)
        nc.vector.tensor_sub(c1[:], t1[:], t2[:])
        nc.vector.reduce_sum(t1[:], e4[:, :, 1, :], axis=Ax.X)
        nc.vector.reduce_sum(t2[:], e4[:, :, 2, :], axis=Ax.X)
        nc.vector.tensor_sub(c2[:], t1[:], t2[:])
        f = pool.tile([P, nj, 7], f32, tag=tg("f"))
        nc.vector.tensor_sub(f[:], e3[:, :, 1:8], e3[:, :, 14:7:-1])
        u1 = pool.tile([P, nj], f32, tag=tg("u1"))
        u2 = pool.tile([P, nj], f32, tag=tg("u2"))
        nc.vector.tensor_sub(u1[:], f[:, :, 0], f[:, :, 1])
        nc.vector.tensor_add(u2[:], f[:, :, 3], f[:, :, 6])
        nc.vector.tensor_sub(u1[:], u1[:], u2[:])
        nc.vector.scalar_tensor_tensor(
            c3[:], f[:, :, 5], -2.0, u1[:], op0=Alu.mult, op1=Alu.add
        )
        for ck in (c0, c1, c2, c3):
            nc.vector.tensor_mul(ck[:], ck[:], r[:])
        return c0, c1, c2, c3

    def combine(a3, b3, cs, hout, vsl, usl, nj, bc, dsA=None, dsB=None,
                nhalf=2):
        """hout = C0 + C1 a + C2 b + C3 ab over [P, nj, bc].

        The DVE runs at ~1 column/cycle per pass regardless of dtype, so
        passes are minimized: ScalarE computes the two per-j affine maps
        v = C3 a + C2 and u = C1 a + C0 (activation with per-partition
        scale+bias, reading a directly), and the DVE needs only
        t = v*b and h = t + u, done per-half so it can start before the
        full v/u chains finish.

        dsA/dsB: DMA-completion semaphores of the (split) prepared gathers
        for the a/b operand halves; attached as waits on the ops that read
        them (a bare engine wait_ge has no data deps, so the Tile scheduler
        is free to hoist it into a deadlock).  v/u only need `a`, so they
        start as soon as the a-half of the gather has landed."""
        c0, c1, c2, c3 = cs
        for j in range(nj):
            iv = nc.scalar.activation(vsl[:, j], a3[:, j], Act.Identity,
                                      bias=c2[:, j:j + 1],
                                      scale=c3[:, j:j + 1])
            if dsA is not None:
                iv.wait_op(dsA, 16, "sem-ge")
            iu = nc.scalar.activation(usl[:, j], a3[:, j], Act.Identity,
                                      bias=c0[:, j:j + 1],
                                      scale=c1[:, j:j + 1])
            if dsA is not None:
                iu.wait_op(dsA, 16, "sem-ge")
        step = (nj + nhalf - 1) // nhalf
        for h0 in range(0, nj, step):
            h1 = min(h0 + step, nj)
            it = nc.vector.tensor_mul(vsl[:, h0:h1], vsl[:, h0:h1],
                                      b3[:, h0:h1])
            if dsB is not None:
                it.wait_op(dsB, 16, "sem-ge")
            nc.vector.tensor_add(hout[:, h0:h1], vsl[:, h0:h1],
                                 usl[:, h0:h1])

    with tile.TileContext(nc) as tc:
        with (
            tc.tile_pool(name="big", bufs=1) as big,
            tc.tile_pool(name="prep", bufs=1) as prep,
            tc.tile_pool(name="small", bufs=2) as small,
        ):
            ds2a = nc.alloc_semaphore("ds2a")
            ds2b = nc.alloc_semaphore("ds2b")
            ds3a = nc.alloc_semaphore("ds3a")
            ds3b = nc.alloc_semaphore("ds3b")

            # shared combine scratch (sliced per layer)
            ct = big.tile([P, NJ3, B], bf16, tag="ct")
            cu = big.tile([P, NJ3, B], bf16, tag="cu")

            # ---- warm-up collective (absorbs first-collective barrier) ----
            wsb = small.tile([1, 16], f32, tag="wsb")
            nc.vector.memset(wsb[:], 0.0)
            nc.sync.dma_start(win[:], wsb[:])
            nc.gpsimd.collective_compute(
                "AllGather", Alu.bypass, replica_groups=G8,
                ins=[win[:]], outs=[warm[:]],
            )

            # ---- L2 gather prep (desc-gen runs now; DMA fires at trigger) ----
            i2t = small.tile([P, 2 * NS12 // 16], i16, tag="i2t")
            nc.sync.dma_start(i2t[:], i2d[:])
            ab2 = big.tile([P, 2 * NJ12, B], hdt, tag="ab2")
            g1rows = gs_[0][:].rearrange("r (j b) -> (r j) b", b=B)
            nc.gpsimd.dma_gather(
                ab2[:, 0:NJ12, :], g1rows, i2t[:, 0:NS12 // 16], NS12, NS12,
                B, prepare_only=True, sem=ds2a, single_packet=SP,
            )
            nc.gpsimd.dma_gather(
                ab2[:, NJ12:2 * NJ12, :], g1rows, i2t[:, NS12 // 16:], NS12,
                NS12, B, prepare_only=True, sem=ds2b, single_packet=SP,
            )

            # ---- layer 1: one-hot matmuls + combine ----
            cs1 = coeffs(prep, wps[0], NJ12, 1)
            h1 = big.tile([P, NJ12, B], hdt, tag="h")

            with tc.tile_pool(name="l1", bufs=1) as l1p:
                xs0 = l1p.tile([P, B], f32, tag="xs0")
                xs1 = l1p.tile([IN - P, B], f32, tag="xs1")
                nc.sync.dma_start(xs0[:], xT[0:P, :])
                nc.sync.dma_start(xs1[:], xT[P:IN, :])
                xb0 = l1p.tile([P, B], hdt, tag="xb0")
                xb1 = l1p.tile([IN - P, B], hdt, tag="xb1")
                nc.scalar.copy(xb0[:], xs0[:])
                nc.scalar.copy(xb1[:], xs1[:])
                oh0t = l1p.tile([P, 2 * NS12], hdt, tag="oh0t")
                oh1t = l1p.tile([IN - P, 2 * NS12], hdt, tag="oh1t")
                nc.sync.dma_start(oh0t[:], oh0[:])
                nc.sync.dma_start(oh1t[:], oh1[:])

                # a/b operand rows land in PSUM chunks; ScalarE computes
                # v = C3 a + C2 and u = C1 a + C0 straight from the a-PSUM
                # (fusing evacuation and affine map in one activation pass),
                # then the DVE finishes h = v*b + u per chunk.
                c0, c1, c2, c3 = cs1
                JC = 2  # j-columns per PSUM chunk (2 banks each for a and b)
                with tc.tile_pool(name="ps1", bufs=2, space="PSUM") as ps1:
                    for jc in range(0, NJ12, JC):
                        pa = ps1.tile([P, JC, B], f32, tag="pa")
                        pb = ps1.tile([P, JC, B], f32, tag="pb")
                        for jj in range(JC):
                            j = jc + jj
                            ca = j * P
                            cb = NS12 + j * P
                            nc.tensor.matmul(pa[:, jj], oh0t[:, ca:ca + P],
                                             xb0[:], start=True, stop=False)
                            nc.tensor.matmul(pa[:, jj], oh1t[:, ca:ca + P],
                                             xb1[:], start=False, stop=True)
                            nc.tensor.matmul(pb[:, jj], oh0t[:, cb:cb + P],
                                             xb0[:], start=True, stop=False)
                            nc.tensor.matmul(pb[:, jj], oh1t[:, cb:cb + P],
                                             xb1[:], start=False, stop=True)
                        for jj in range(JC):
                            j = jc + jj
                            nc.scalar.activation(ct[:, j], pa[:, jj],
                                                 Act.Identity,
                                                 bias=c2[:, j:j + 1],
                                                 scale=c3[:, j:j + 1])
                            nc.scalar.activation(cu[:, j], pa[:, jj],
                                                 Act.Identity,
                                                 bias=c0[:, j:j + 1],
                                                 scale=c1[:, j:j + 1])
                        nc.vector.tensor_mul(ct[:, jc:jc + JC],
                                             ct[:, jc:jc + JC], pb[:])
                        nc.vector.tensor_add(h1[:, jc:jc + JC, :],
                                             ct[:, jc:jc + JC],
                                             cu[:, jc:jc + JC])
                        if jc + JC in (JH, NJ12):
                            k = (jc + JC) // JH - 1
                            nc.sync.dma_start(
                                cin[0][k][:],
                                h1[:, k * JH:(k + 1) * JH, :].rearrange(
                                    "p j b -> p (j b)"))
                            nc.gpsimd.collective_compute(
                                "AllGather", Alu.bypass, replica_groups=G8,
                                ins=[cin[0][k][:]],
                                outs=[gs_[0][k * SH * P:(k + 1) * SH * P, :]],
                            )

            # ---- trigger L2 gather once both g1 halves have landed ----
            pr1 = small.tile([2, 64], hdt, tag="pr1")
            nc.sync.dma_start(pr1[0:1, :], gs_[0][0:1, 0:64])
            nc.sync.dma_start(pr1[1:2, :], gs_[0][SH * P:SH * P + 1, 0:64])
            nc.gpsimd.trigger_dma(count=None, signals_writable=[pr1[:], ab2[:]])

            # ---- L3 gather prep (desc-gen after trigger2 in program order) ----
            i3t = small.tile([P, 2 * NS3 // 16], i16, tag="i3t")
            nc.sync.dma_start(i3t[:], i3d[:])
            ab3 = big.tile([P, 2 * NJ3, B], hdt, tag="ab3")
            g2rows = gs_[1][:].rearrange("r (j b) -> (r j) b", b=B)
            nc.gpsimd.dma_gather(
                ab3[:, 0:NJ3, :], g2rows, i3t[:, 0:NS3 // 16], NS3, NS3,
                B, prepare_only=True, sem=ds3a, single_packet=SP,
            )
            nc.gpsimd.dma_gather(
                ab3[:, NJ3:2 * NJ3, :], g2rows, i3t[:, NS3 // 16:], NS3,
                NS3, B, prepare_only=True, sem=ds3b, single_packet=SP,
            )

            # ---- layer 2 ----
            cs2 = coeffs(prep, wps[1], NJ12, 2)
            h2 = big.tile([P, NJ12, B], hdt, tag="h")
            combine(ab2[:, 0:NJ12, :], ab2[:, NJ12:2 * NJ12, :], cs2, h2[:],
                    ct[:, 0:NJ12, :], cu[:, 0:NJ12, :], NJ12, B,
                    dsA=ds2a, dsB=ds2b)
            for k in range(2):
                nc.sync.dma_start(
                    cin[1][k][:],
                    h2[:, k * JH:(k + 1) * JH, :].rearrange("p j b -> p (j b)"))
                nc.gpsimd.collective_compute(
                    "AllGather", Alu.bypass, replica_groups=G8,
                    ins=[cin[1][k][:]],
                    outs=[gs_[1][k * SH * P:(k + 1) * SH * P, :]],
                )

            # ---- trigger L3 gather ----
            pr2 = small.tile([2, 64], hdt, tag="pr2")
            nc.sync.dma_start(pr2[0:1, :], gs_[1][0:1, 0:64])
            nc.sync.dma_start(pr2[1:2, :], gs_[1][SH * P:SH * P + 1, 0:64])
            nc.gpsimd.trigger_dma(count=None, signals_writable=[pr2[:], ab3[:]])

            # ---- layer 3 ----
            cs3 = coeffs(prep, wps[2], NJ3, 3)
            h3 = big.tile([P, NJ3, B], bf16, tag="h3")
            combine(ab3[:, 0:NJ3, :], ab3[:, NJ3:2 * NJ3, :], cs3, h3[:],
                    ct[:], cu[:], NJ3, B, dsA=ds3a, dsB=ds3b, nhalf=3)

            # ---- GroupSum ----
            gsum = prep.tile([P, NGROUP * B], f32, tag="gsum")
            for g in range(NGROUP):
                sl = h3[:, g * JPG:(g + 1) * JPG, :].rearrange("p j b -> p b j")
                nc.vector.reduce_sum(gsum[:, g * B:(g + 1) * B], sl, axis=Ax.X)
            ones = small.tile([P, 1], f32, tag="ones")
            nc.vector.memset(ones[:], 1.0)
            psc = prep.tile([1, NGROUP * B], f32, tag="psc")
            with tc.tile_pool(name="ps2", bufs=2, space="PSUM") as ps2:
                for g in range(NGROUP):
                    ps = ps2.tile([1, B], f32, tag="psg")
                    nc.tensor.matmul(ps[:], ones[:], gsum[:, g * B:(g + 1) * B],
                                     start=True, stop=True)
                    nc.scalar.copy(psc[:, g * B:(g + 1) * B], ps[:])
                nc.sync.dma_start(pin[:], psc[:])
                nc.gpsimd.collective_compute(
                    "AllGather", Alu.bypass, replica_groups=G8,
                    ins=[pin[:]], outs=[pall[:]],
                )
                pall_sb = prep.tile([8, NGROUP * B], f32, tag="pall_sb")
                nc.sync.dma_start(pall_sb[:], pall[:])
                ones8 = small.tile([8, 1], f32, tag="ones8")
                nc.vector.memset(ones8[:], 1.0)
                osb = prep.tile([1, NGROUP * B], f32, tag="osb")
                for g in range(NGROUP):
                    ps2t = ps2.tile([1, B], f32, tag="psg2")
                    nc.tensor.matmul(ps2t[:], ones8[:],
                                     pall_sb[:, g * B:(g + 1) * B],
                                     start=True, stop=True)
                    nc.scalar.mul(osb[:, g * B:(g + 1) * B], ps2t[:], 1.0 / TAU)
            # consume the warm-up collective's (all-zero) output
            wsb2 = small.tile([1, 16], f32, tag="wsb2")
            nc.sync.dma_start(wsb2[:], warm[0:1, :])
            nc.vector.tensor_add(osb[:, :16], osb[:, :16], wsb2[:])
            nc.sync.dma_start(out_d[:], osb[:])

    nc.compile()
    return nc


def _wrap_idx(ii):
    w = ii.astype(np.int16).reshape(-1, 16).T
    return np.ascontiguousarray(np.tile(w, (8, 1)))


JH = NJ12 // 2


def _src_unit(i):
    """Row unit of layer-1/2 neuron i in the chunk-major AllGathered
    [2*SH*128, JH*B] layout: shard s = i//2000, local t = i - 2000s,
    p = t%128, j = t//128, half k = j//JH; row = k*SH*128 + s*128 + p,
    unit = row*JH + j%JH."""
    s = i // REAL12
    t = i - s * REAL12
    p = t % P
    j = t // P
    k = j // JH
    return ((k * SH + s) * P + p) * JH + j % JH


def _pack_w(w_eff, nj):
    # local slot t = j*128 + p  ->  packed[p, j*16+g]
    return np.ascontiguousarray(
        w_eff.reshape(nj, P, 16).transpose(1, 0, 2).reshape(P, nj * 16)
    )


def _host_pack(inputs):
    x = np.asarray(inputs["x"], dtype=np.float32)
    w1 = np.asarray(inputs["w1"], dtype=np.float32)
    w2 = np.asarray(inputs["w2"], dtype=np.float32)
    w3 = np.asarray(inputs["w3"], dtype=np.float32)
    i1a = np.asarray(inputs["idx1a"]).astype(np.int64)
    i1b = np.asarray(inputs["idx1b"]).astype(np.int64)
    i2a = np.asarray(inputs["idx2a"]).astype(np.int64)
    i2b = np.asarray(inputs["idx2b"]).astype(np.int64)
    i3a = np.asarray(inputs["idx3a"]).astype(np.int64)
    i3b = np.asarray(inputs["idx3b"]).astype(np.int64)

    import ml_dtypes

    pad_row = np.full(16, -20.0, dtype=np.float32)
    pad_row[0] = 20.0  # softmax -> ~one-hot FALSE gate -> h = 0

    xTf = np.ascontiguousarray(x.T)  # [193, 512]

    per_shard = []
    for s in range(SH):
        m = {}
        sel = slice(s * REAL12, (s + 1) * REAL12)

        # layer 1: one-hot matrices + packed weights
        w1_eff = np.concatenate(
            [w1[sel], np.tile(pad_row, (NS12 - REAL12, 1))], axis=0
        )
        m["w1p"] = _pack_w(w1_eff, NJ12)
        oh = np.zeros((IN, 2 * NS12),
                      dtype=(ml_dtypes.float8_e4m3 if HD == "f8"
                             else ml_dtypes.bfloat16))
        cols = np.arange(REAL12)
        oh[i1a[sel], cols] = 1.0
        oh[i1b[sel], NS12 + cols] = 1.0
        m["oh0"] = np.ascontiguousarray(oh[0:P])
        m["oh1"] = np.ascontiguousarray(oh[P:IN])

        # layer 2
        w2_eff = np.concatenate(
            [w2[sel], np.tile(pad_row, (NS12 - REAL12, 1))], axis=0
        )
        m["w2p"] = _pack_w(w2_eff, NJ12)
        ia = np.zeros(NS12, dtype=np.int64)
        ib = np.zeros(NS12, dtype=np.int64)
        ia[:REAL12] = _src_unit(i2a[sel])
        ib[:REAL12] = _src_unit(i2b[sel])
        m["i2"] = _wrap_idx(np.concatenate([ia, ib]))

        # layer 3: group g's 5333 real neurons split per CNT3; within (s, g):
        # local j in [6g, 6g+6), rank m = (j-6g)*128 + p
        u = np.arange(NS3)
        jj = u // P
        pp = u % P
        gg = jj // JPG
        mm = (jj - gg * JPG) * P + pp
        real = mm < CNT3[s]
        rid = gg * SPG + OFF3[s] + np.minimum(mm, CNT3[s] - 1)
        w3_eff = w3[rid].copy()
        w3_eff[~real] = pad_row
        m["w3p"] = _pack_w(w3_eff, NJ3)
        i3a_eff = np.where(real, _src_unit(i3a[rid]), 0)
        i3b_eff = np.where(real, _src_unit(i3b[rid]), 0)
        m["i3"] = _wrap_idx(np.concatenate([i3a_eff, i3b_eff]))

        m["xT"] = xTf
        per_shard.append(m)
    return per_shard


LAST_RESULTS = None


def kernel(**inputs):
    global LAST_RESULTS
    from concourse.bass_utils import run_bass_kernel_spmd

    if "nc" not in _CACHE:
        _CACHE["nc"] = _build_nc()
    nc = _CACHE["nc"]

    in_maps = _host_pack(inputs)
    trace = bool(int(os.environ.get("KERNEL_TRACE", "0")))
    res = run_bass_kernel_spmd(
        nc, in_maps, core_ids=list(range(N_CORES)), trace=trace
    )
    LAST_RESULTS = res

    rc = res.results[0]["out"].reshape(NGROUP, B)
    return np.ascontiguousarray(rc.T.astype(np.float32))
